# Initial kernel scaffold
#
"""RNN-T joint network (Conformer transducer) kernel for Trainium2.

Computes out[b,t,u,v] = (enc[b,t,:] @ W[:, :D].T)[v] + (dec[b,u,:] @ W[:, D:].T)[v]
i.e. the broadcast-sum decomposition of cat(enc, dec) @ W.T without
materialising the (B,T,U,2D) concat.

Sharding: the (B*T) = 1024 grid rows are split across 8 NeuronCores
(cores 0-3 take b=0, cores 4-7 take b=1, 128 t-rows each). W is
replicated. Each core emits its own (128, U, V) fp32 slab (64 MB); the
host reassembles the full (B,T,U,V) tensor.

Per-core structure (exact to ~1e-6 of a straight fp32 impl):
  1. enc_proj = encT.T @ W_encT  and  dec_proj = decT.T @ W_decT on the
     TensorEngine (fp32 matmuls, K=512 in 4 chunks). Each K-chunk's
     lhsT and rhs live in one packed SBUF tile fed by a single DMA, so
     every matmul carries at most one sync wait (walrus LDWEIGHTS limit).
  2. enc_proj is split into an fp16 hi/lo pair (hi = fp16(x),
     lo = fp16(x - hi)); hi + lo reconstructs x to ~2^-22 relative.
  3. For each t: a one-hot fp16 "selector" matmul broadcasts row t of
     enc_hi (then enc_lo, accumulated) across all 128 PSUM partitions.
     Matmul cost is N cycles regardless of K, so this is ~4x cheaper
     than an fp32 broadcast matmul.
  4. VectorEngine adds dec_proj (fp32, SBUF) to the PSUM broadcast and
     writes the (128u, 1024v) output tile to SBUF.
  5. HWDGE DMA streams each 512 KB tile to DRAM (contiguous).

The kernel is DMA-bound: 64 MB of output per core at ~360 GB/s/core.
"""

import numpy as np

import concourse.bass as bass
import concourse.tile as tile
from concourse import bacc
from concourse import mybir
from concourse.bass_utils import run_bass_kernel_spmd

B, T, U, D, V = 2, 512, 128, 512, 1024
N_CORES = 8
T_LOC = (B * T) // N_CORES  # 128 t-rows per core
PKW = 128 + V  # packed chunk width: [lhsT column block | rhs row block]

F32 = mybir.dt.float32
F16 = mybir.dt.float16


def _build_program() -> bass.Bass:
    nc = bacc.Bacc("TRN2", debug=False, num_devices=N_CORES)

    # PACK[kc] = [encT chunk kc | WT chunk kc]        for kc in 0..3
    #          = [decT chunk kc-4 | WT chunk kc]      for kc in 4..7
    PACK = nc.dram_tensor("PACK", [8, 128, PKW], F32, kind="ExternalInput").ap()
    SELR = nc.dram_tensor("SELR", [128, 32 * 128], F16, kind="ExternalInput").ap()
    OUT = nc.dram_tensor("out", [T_LOC, U, V], F32, kind="ExternalOutput").ap()

    with tile.TileContext(nc) as tc:
        with (
            tc.tile_pool(name="const", bufs=1) as cpool,
            tc.tile_pool(name="pmain", bufs=2, space="PSUM") as pmain,
            tc.tile_pool(name="outp", bufs=8) as opool,
        ):
            # ---- inputs to SBUF ----
            sel_raw = cpool.tile([128, 32 * 128], F16, tag="selraw")
            nc.sync.dma_start(out=sel_raw[:], in_=SELR)

            # dec chunks (4-7) first: the dec projection runs first on the PE.
            pk = [None] * 8
            for kc in (4, 5, 6, 7, 0, 1, 2, 3):
                tl = cpool.tile([128, PKW], F32, tag=f"pk{kc}")
                nc.sync.dma_start(out=tl[:], in_=PACK[kc])
                pk[kc] = tl

            # Re-materialise sel via the VectorEngine so the selector
            # matmuls' dependencies (sel, enc_hi, enc_lo) all resolve to a
            # single DVE semaphore wait.
            sel = cpool.tile([128, 32 * 128], F16, tag="sel")
            nc.vector.tensor_copy(out=sel[:], in_=sel_raw[:])

            # ---- dec_proj = decT.T @ W_decT : (U, V) ----
            # dec first: its DVE copies then overlap the enc matmuls, so the
            # first main-loop add is gated only by the enc cast chain.
            # Projections borrow the main-loop PSUM slots (4 banks each).
            dec_ps = pmain.tile([128, 2 * V], F32, tag="ps")
            for vh in range(2):
                for kc in range(4):
                    nc.tensor.matmul(
                        dec_ps[:, 512 * vh : 512 * (vh + 1)],
                        lhsT=pk[4 + kc][:, 0:128],
                        rhs=pk[4 + kc][:, 128 + 512 * vh : 128 + 512 * (vh + 1)],
                        start=(kc == 0),
                        stop=(kc == 3),
                    )
            # dec_proj duplicated side by side so a single FD=2048 DVE add
            # covers a pair of t-tiles.
            dec2 = cpool.tile([128, 2 * V], F32, tag="dec2")
            nc.vector.tensor_copy(out=dec2[:, 0:V], in_=dec_ps[:, 0:V])
            nc.vector.tensor_copy(out=dec2[:, V : 2 * V], in_=dec_ps[:, 0:V])

            # ---- enc_proj = encT.T @ W_encT : (T_LOC, V) ----
            enc_ps = pmain.tile([128, 2 * V], F32, tag="ps")
            for vh in range(2):
                for kc in range(4):
                    nc.tensor.matmul(
                        enc_ps[:, 512 * vh : 512 * (vh + 1)],
                        lhsT=pk[kc][:, 0:128],
                        rhs=pk[kc][:, 128 + 512 * vh : 128 + 512 * (vh + 1)],
                        start=(kc == 0),
                        stop=(kc == 3),
                    )
            enc_hi = cpool.tile([128, V], F16, tag="ehi")
            enc_lo = cpool.tile([128, V], F16, tag="elo")
            nc.vector.tensor_copy(out=enc_hi[:], in_=enc_ps[:, 0:V])
            nc.vector.tensor_sub(out=enc_lo[:], in0=enc_ps[:, 0:V], in1=enc_hi[:])

            # ---- main loop: two (128u, 1024v) output tiles per unit ----
            # j-outer / gp-inner; each unit covers t0 = 32*gp + j and
            # t1 = 32*(gp+1) + j. Matmul order alternates PSUM banks
            # (vh0/vh1) so fills overlap drains, and alternates PE row
            # groups across g so weight loads overlap running matmuls.
            for j in range(32):
                for gp in (0, 2):
                    ps = pmain.tile([128, 2 * V], F32, tag="ps")
                    ob = opool.tile([128, 2 * V], F32, tag="ob")
                    for gg in range(2):
                        g = gp + gg
                        sel_ap = sel[32 * g : 32 * (g + 1), 128 * j : 128 * (j + 1)]
                        for src, is_hi in ((enc_hi, True), (enc_lo, False)):
                            for vh in range(2):
                                lo, hi = 512 * vh, 512 * (vh + 1)
                                nc.tensor.matmul(
                                    ps[:, V * gg + lo : V * gg + hi],
                                    lhsT=sel_ap,
                                    rhs=src[32 * g : 32 * (g + 1), lo:hi],
                                    start=is_hi,
                                    stop=not is_hi,
                                    tile_position=(32 * g, 0),
                                    skip_group_check=True,
                                )
                    nc.vector.tensor_add(out=ob[:], in0=ps[:], in1=dec2[:])
                    nc.sync.dma_start(out=OUT[32 * gp + j], in_=ob[:, 0:V])
                    nc.sync.dma_start(out=OUT[32 * (gp + 1) + j], in_=ob[:, V : 2 * V])
    nc.compile()
    return nc


def _build_sel() -> np.ndarray:
    # SEL[k, 128*j + u] = 1 iff j == k % 32: slicing columns [128j, 128j+128)
    # of partition rows [32g, 32g+32) yields the one-hot matrix that picks
    # row 32g+j of the rhs and replicates it across all 128 output partitions.
    sel = np.zeros((128, 32 * 128), np.float16)
    for k in range(128):
        j = k % 32
        sel[k, 128 * j : 128 * (j + 1)] = 1.0
    return sel


_PROGRAM = None


def _get_program() -> bass.Bass:
    global _PROGRAM
    if _PROGRAM is None:
        _PROGRAM = _build_program()
    return _PROGRAM


def _make_in_maps(inputs):
    enc = np.asarray(inputs["encoder_outputs"], dtype=np.float32)
    dec = np.asarray(inputs["decoder_outputs"], dtype=np.float32)
    W = np.asarray(inputs["W"], dtype=np.float32)
    WT = np.ascontiguousarray(W.T)  # (2D, V)
    SEL = _build_sel()
    in_maps = []
    for c in range(N_CORES):
        b = c // (N_CORES // B)
        t0 = (c % (N_CORES // B)) * T_LOC
        encT = enc[b, t0 : t0 + T_LOC, :].T  # (D, T_LOC)
        decT = dec[b].T  # (D, U)
        pack = np.empty((8, 128, PKW), np.float32)
        for kc in range(4):
            pack[kc, :, :128] = encT[128 * kc : 128 * (kc + 1), :]
            pack[kc, :, 128:] = WT[128 * kc : 128 * (kc + 1), :]
        for kc in range(4, 8):
            pack[kc, :, :128] = decT[128 * (kc - 4) : 128 * (kc - 3), :]
            pack[kc, :, 128:] = WT[128 * kc : 128 * (kc + 1), :]
        in_maps.append({"PACK": pack, "SELR": SEL})
    return in_maps


def _assemble(results) -> np.ndarray:
    out = np.empty((B, T, U, V), np.float32)
    for c in range(N_CORES):
        b = c // (N_CORES // B)
        t0 = (c % (N_CORES // B)) * T_LOC
        out[b, t0 : t0 + T_LOC] = results[c]["out"]
    return out


def _run(inputs, **spmd_kwargs):
    nc = _get_program()
    in_maps = _make_in_maps(inputs)
    res = run_bass_kernel_spmd(nc, in_maps, core_ids=list(range(N_CORES)), **spmd_kwargs)
    return _assemble(res.results), res


def kernel(**inputs) -> np.ndarray:
    out, _ = _run(inputs)
    return out



# revision 1
# speedup vs baseline: 1.0361x; 1.0361x over previous
"""RNN-T joint network (Conformer transducer) kernel for Trainium2.

Computes out[b,t,u,v] = (enc[b,t,:] @ W[:, :D].T)[v] + (dec[b,u,:] @ W[:, D:].T)[v]
i.e. the broadcast-sum decomposition of cat(enc, dec) @ W.T without
materialising the (B,T,U,2D) concat.

Sharding: the (B*T) = 1024 grid rows are split across 8 NeuronCores
(cores 0-3 take b=0, cores 4-7 take b=1, 128 t-rows each). W is
replicated. Each core emits its own (128, U, V) fp32 slab (64 MB); the
host reassembles the full (B,T,U,V) tensor.

Per-core structure (exact to ~1e-6 of a straight fp32 impl):
  1. enc_proj = encT.T @ W_encT  and  dec_proj = decT.T @ W_decT on the
     TensorEngine (fp32 matmuls, K=512 in 4 chunks). Each K-chunk's
     lhsT and rhs live in one packed SBUF tile fed by a single DMA, so
     every matmul carries at most one sync wait (walrus LDWEIGHTS limit).
  2. enc_proj is split into an fp16 hi/lo pair (hi = fp16(x),
     lo = fp16(x - hi)); hi + lo reconstructs x to ~2^-22 relative.
  3. For each t: a one-hot fp16 "selector" matmul broadcasts row t of
     enc_hi (then enc_lo, accumulated) across all 128 PSUM partitions.
     Matmul cost is N cycles regardless of K, so this is ~4x cheaper
     than an fp32 broadcast matmul.
  4. VectorEngine adds dec_proj (fp32, SBUF) to the PSUM broadcast and
     writes the (128u, 1024v) output tile to SBUF.
  5. HWDGE DMA streams each 512 KB tile to DRAM (contiguous).

The kernel is DMA-bound: 64 MB of output per core at ~360 GB/s/core.
"""

import numpy as np

import concourse.bass as bass
import concourse.tile as tile
from concourse import bacc
from concourse import mybir
from concourse.bass_utils import run_bass_kernel_spmd

B, T, U, D, V = 2, 512, 128, 512, 1024
N_CORES = 8
T_LOC = (B * T) // N_CORES  # 128 t-rows per core
PKW = 128 + V  # packed chunk width: [lhsT column block | rhs row block]

F32 = mybir.dt.float32
F16 = mybir.dt.float16


def _build_program() -> bass.Bass:
    nc = bacc.Bacc("TRN2", debug=False, num_devices=N_CORES)

    # PACK[kc] = [encT chunk kc | WT chunk kc]        for kc in 0..3
    #          = [decT chunk kc-4 | WT chunk kc]      for kc in 4..7
    PACK = nc.dram_tensor("PACK", [8, 128, PKW], F32, kind="ExternalInput").ap()
    SELR = nc.dram_tensor("SELR", [128, 32 * 128], F16, kind="ExternalInput").ap()
    OUT = nc.dram_tensor("out", [T_LOC, U, V], F32, kind="ExternalOutput").ap()

    with tile.TileContext(nc) as tc:
        with (
            tc.tile_pool(name="const", bufs=1) as cpool,
            tc.tile_pool(name="pmain", bufs=2, space="PSUM") as pmain,
            tc.tile_pool(name="outp", bufs=8) as opool,
        ):
            # ---- inputs to SBUF ----
            sel_raw = cpool.tile([128, 32 * 128], F16, tag="selraw")
            nc.sync.dma_start(out=sel_raw[:], in_=SELR)

            # dec chunks (4-7) first: the dec projection runs first on the PE.
            pk = [None] * 8
            for kc in (4, 5, 6, 7, 0, 1, 2, 3):
                tl = cpool.tile([128, PKW], F32, tag=f"pk{kc}")
                nc.sync.dma_start(out=tl[:], in_=PACK[kc])
                pk[kc] = tl

            # Re-materialise sel via the VectorEngine so the selector
            # matmuls' dependencies (sel, enc_hi, enc_lo) all resolve to a
            # single DVE semaphore wait.
            sel = cpool.tile([128, 32 * 128], F16, tag="sel")
            nc.vector.tensor_copy(out=sel[:], in_=sel_raw[:])

            # ---- dec_proj = decT.T @ W_decT : (U, V) ----
            # dec first: its DVE copies then overlap the enc matmuls, so the
            # first main-loop add is gated only by the enc cast chain.
            # Projections borrow the main-loop PSUM slots (4 banks each).
            dec_ps = pmain.tile([128, 2 * V], F32, tag="ps")
            for vh in range(2):
                for kc in range(4):
                    nc.tensor.matmul(
                        dec_ps[:, 512 * vh : 512 * (vh + 1)],
                        lhsT=pk[4 + kc][:, 0:128],
                        rhs=pk[4 + kc][:, 128 + 512 * vh : 128 + 512 * (vh + 1)],
                        start=(kc == 0),
                        stop=(kc == 3),
                    )
            # dec_proj duplicated side by side so a single FD=2048 DVE add
            # covers a pair of t-tiles.
            dec2 = cpool.tile([128, 2 * V], F32, tag="dec2")
            nc.vector.tensor_copy(out=dec2[:, 0:V], in_=dec_ps[:, 0:V])
            nc.vector.tensor_copy(out=dec2[:, V : 2 * V], in_=dec_ps[:, 0:V])

            # ---- enc_proj = encT.T @ W_encT : (T_LOC, V) ----
            enc_ps = pmain.tile([128, 2 * V], F32, tag="ps")
            for vh in range(2):
                for kc in range(4):
                    nc.tensor.matmul(
                        enc_ps[:, 512 * vh : 512 * (vh + 1)],
                        lhsT=pk[kc][:, 0:128],
                        rhs=pk[kc][:, 128 + 512 * vh : 128 + 512 * (vh + 1)],
                        start=(kc == 0),
                        stop=(kc == 3),
                    )
            enc_hi = cpool.tile([128, V], F16, tag="ehi")
            enc_lo = cpool.tile([128, V], F16, tag="elo")
            nc.vector.tensor_copy(out=enc_hi[:], in_=enc_ps[:, 0:V])
            nc.vector.tensor_sub(out=enc_lo[:], in0=enc_ps[:, 0:V], in1=enc_hi[:])

            # ---- main loop: two (128u, 1024v) output tiles per unit ----
            # j-outer / gp-inner; each unit covers t0 = 32*gp + j and
            # t1 = 32*(gp+1) + j. Matmul order alternates PSUM banks
            # (vh0/vh1) so fills overlap drains, and alternates PE row
            # groups across g so weight loads overlap running matmuls.
            for j in range(32):
                for gp in (0, 2):
                    ps = pmain.tile([128, 2 * V], F32, tag="ps")
                    ob = opool.tile([128, 2 * V], F32, tag="ob")
                    for gg in range(2):
                        g = gp + gg
                        sel_ap = sel[32 * g : 32 * (g + 1), 128 * j : 128 * (j + 1)]
                        for src, is_hi in ((enc_hi, True), (enc_lo, False)):
                            for vh in range(2):
                                lo, hi = 512 * vh, 512 * (vh + 1)
                                nc.tensor.matmul(
                                    ps[:, V * gg + lo : V * gg + hi],
                                    lhsT=sel_ap,
                                    rhs=src[32 * g : 32 * (g + 1), lo:hi],
                                    start=is_hi,
                                    stop=not is_hi,
                                    tile_position=(32 * g, 0),
                                    skip_group_check=True,
                                )
                    nc.vector.tensor_add(out=ob[:], in0=ps[:], in1=dec2[:])
                    nc.sync.dma_start(out=OUT[32 * gp + j], in_=ob[:, 0:V])
                    nc.sync.dma_start(out=OUT[32 * (gp + 1) + j], in_=ob[:, V : 2 * V])
    nc.compile()
    return nc


def _build_sel() -> np.ndarray:
    # SEL[k, 128*j + u] = 1 iff j == k % 32: slicing columns [128j, 128j+128)
    # of partition rows [32g, 32g+32) yields the one-hot matrix that picks
    # row 32g+j of the rhs and replicates it across all 128 output partitions.
    sel = np.zeros((128, 32 * 128), np.float16)
    for k in range(128):
        j = k % 32
        sel[k, 128 * j : 128 * (j + 1)] = 1.0
    return sel


_PROGRAM = None


def _get_program() -> bass.Bass:
    global _PROGRAM
    if _PROGRAM is None:
        _PROGRAM = _build_program()
    return _PROGRAM


def _make_in_maps(inputs):
    enc = np.asarray(inputs["encoder_outputs"], dtype=np.float32)
    dec = np.asarray(inputs["decoder_outputs"], dtype=np.float32)
    W = np.asarray(inputs["W"], dtype=np.float32)
    WT = np.ascontiguousarray(W.T)  # (2D, V)
    SEL = _build_sel()
    in_maps = []
    for c in range(N_CORES):
        b = c // (N_CORES // B)
        t0 = (c % (N_CORES // B)) * T_LOC
        encT = enc[b, t0 : t0 + T_LOC, :].T  # (D, T_LOC)
        decT = dec[b].T  # (D, U)
        pack = np.empty((8, 128, PKW), np.float32)
        for kc in range(4):
            pack[kc, :, :128] = encT[128 * kc : 128 * (kc + 1), :]
            pack[kc, :, 128:] = WT[128 * kc : 128 * (kc + 1), :]
        for kc in range(4, 8):
            pack[kc, :, :128] = decT[128 * (kc - 4) : 128 * (kc - 3), :]
            pack[kc, :, 128:] = WT[128 * kc : 128 * (kc + 1), :]
        in_maps.append({"PACK": pack, "SELR": SEL})
    return in_maps


def _assemble(results) -> np.ndarray:
    out = np.empty((B, T, U, V), np.float32)
    for c in range(N_CORES):
        b = c // (N_CORES // B)
        t0 = (c % (N_CORES // B)) * T_LOC
        out[b, t0 : t0 + T_LOC] = results[c]["out"]
    return out


def _run(inputs, **spmd_kwargs):
    nc = _get_program()
    in_maps = _make_in_maps(inputs)
    res = run_bass_kernel_spmd(nc, in_maps, core_ids=list(range(N_CORES)), **spmd_kwargs)
    return _assemble(res.results), res


def kernel(**inputs) -> np.ndarray:
    out, _ = _run(inputs)
    return out

